# revision 30
# baseline (speedup 1.0000x reference)
"""Trainium2 Bass kernel for nn_CustomParameterTransform (scatter_memory).

Reference semantics: coord_v [256, 30] holds 10 (x, y, mass) triplets per
sample. Each triplet maps to integer grid indices (x_i, y_i, m_i); a one-hot
volume z [B, 16, 128, 128] is scattered (z[b, m, y, x] = 1) and the output is
concat(1-z, z) over the channel axis -> [256, 32, 128, 128] f32 (512 MB).

Strategy (8 NeuronCores, batch-sharded, no cross-core comm):
  - The output is almost entirely constant (first 16 channels 1.0, last 16
    0.0, except at 640 scatter points per core). Per core: one 64 MB
    write-only region built from SBUF "slab image" tiles whose
    partition-major sweep reproduces whole slabs (alternating 1 MB ones /
    1 MB zeros), so every fill is a contiguous DRAM write and both DMA
    sides stay 2-D (the HWDGE PDMA2D fast path; 3-D/strided APs demote to
    an engine-sequenced slow path measured ~5x slower).
  - Steady-state throughput is capped by the per-core DMA port (~435
    GB/s); a ring dispatches ~4 descriptors/us per outstanding
    instruction, so the ramp is limited by how quickly fill instructions
    become ready. Hence: a [128, 1024] mini tile (one ~0.9 us memset per
    engine) feeds the very first fills, slabs 1-5 are ten 1 MB half-slab
    fills (lots of outstanding instructions early), and slabs 6+ are 4 MB
    fills from a [128, 8192] tile (32 KB rows; rows can't exceed 32 KB
    because a larger slab image needs its value to alternate every <32
    partitions and compute APs must start on 32-partition quadrant
    boundaries).
  - A gpsimd software-DGE fill queue was tried as a third descriptor
    stream and made things worse: engines stall fetching software
    descriptors, throttling the HWDGE rings. Everything stays on the two
    rings.
  - The 640 scatter points are fixed up with indirect (scatter) DMAs whose
    deps are wired to just the fills covering their samples, so all but
    the last column overlap the fill phase.
  - The stock const-AP all-engine barrier in Bass.__init__ is patched out
    (nothing here uses const_aps) and TileContext's epilogue is replaced
    with a light drain, since the event-lowered sem-clear cascade scales
    with instruction count.
  - Indices are computed on the host with the exact same jax ops as the
    reference (bit-identical floor/log10 behavior) and passed per-core as
    a [128, 5] int32 tensor of flat element offsets.
"""

import numpy as np

B = 256
NSRC = 10
NMC = 16
L = 128
NCORES = 8
BL = B // NCORES          # 32 samples per core
PLANE = L * L             # 16384
HALF = NMC * PLANE        # 262144 elements per half-slab
SLAB = 2 * HALF           # 524288 elements per sample
OUT_ELEMS = BL * SLAB     # 16777216 per core (64 MB)

N_SCATTER_COLS = 5        # 640 scatter writes = 128 partitions x 5 columns
PTS = BL * NSRC           # 320 points per core

_CACHE = {}


def _build_nc():
    import concourse.bass as bass
    import concourse.tile as tile
    from concourse import bacc, mybir
    from concourse.tile_rust import add_dep_helper

    import types as _types
    from concourse.vector_clock import ScopedClock

    # The const-AP registration in Bass.__init__ ends with an all-engine
    # barrier (~1.5 us of event-sem chaining at the head of every
    # execution). This kernel never touches const_aps -- memset packs its
    # immediate and the DMAs don't use them -- so elide the barrier for
    # the duration of construction.
    _orig_barrier = bass.Bass.all_engine_barrier
    bass.Bass.all_engine_barrier = lambda self, **kw: None
    try:
        nc = bacc.Bacc("TRN2", target_bir_lowering=False, debug=False,
                       num_devices=NCORES)
    finally:
        bass.Bass.all_engine_barrier = _orig_barrier

    def _light_drain_and_barrier(self, tick_clock, wait_clock):
        """Replaces TileContext._drain_and_barrier for this kernel. The
        stock epilogue is drain + two all-engine EVSEM butterfly barriers
        around the sem clear (~9 us after event lowering). Requirements at
        kernel end are: (1) all DMA completions observed, (2) sems cleared
        for NEFF re-execution, (3) the clear happens after every engine's
        last sem use. (1) is the sync drain's global-clock waits; (3) is a
        counting-sem join (sync arrives only after the drain, so join>=4
        implies all DMA done); (2) is the ranged clear. The second barrier
        is unnecessary: a re-execution cannot start until every engine --
        including the clearing gpsimd -- has ended."""
        nc_ = self.nc
        drain_inst = nc_.sync.drain()
        wait_clock.add_sem_waits(
            drain_inst.ins, ScopedClock({None: tick_clock.global_clock}))
        join = nc_.alloc_semaphore("tail_join")
        for eng in nc_.engines.values():
            if eng is not nc_.gpsimd:
                eng.sem_inc(join, 1)
        n_other = len(nc_.engines) - 1
        nc_.gpsimd.wait_ge(join, n_other)
        popped = nc_._tile_sem_poison_stack.pop()
        assert popped is self._sem_poison
        sems = list(self.sems.allocated().values())
        nc_.clear_and_free_semaphores(sems + [join])

    offs = nc.dram_tensor("offs", [128, N_SCATTER_COLS], mybir.dt.int32,
                          kind="ExternalInput").ap()
    out = nc.dram_tensor("out", [BL, SLAB], mybir.dt.float32,
                         kind="ExternalOutput").ap()

    with tile.TileContext(nc) as tc:
        tc._drain_and_barrier = _types.MethodType(_light_drain_and_barrier, tc)
        with tc.tile_pool(name="src", bufs=1) as src_pool, \
             tc.tile_pool(name="small", bufs=1) as small_pool:
            # Stage A [128, 4096] (1 slab/sweep, 16 KB rows). The first
            # memset on each engine yields one CONSTANT quarter (vector:
            # ones in rows 0-63 cols 0-2048; gpsimd: zeros in rows 64-127
            # cols 2048-4096) at ~7.4 us -- the two rings' first fills
            # read those opposite-partition quarters so both rings start
            # immediately on disjoint engine groups. The full slab image
            # is ready ~9 us.
            slab_a = src_pool.tile([128, 4096], mybir.dt.float32)
            nc.vector.memset(slab_a[0:64, 0:2048], 1.0)
            nc.gpsimd.memset(slab_a[64:128, 2048:4096], 0.0)
            nc.vector.memset(slab_a[64:128, 0:2048], 0.0)
            nc.gpsimd.memset(slab_a[0:64, 2048:4096], 1.0)
            # Stage B [128, 8192] (2 slabs/sweep, 32 KB rows, value
            # alternating every 32 rows), columns split vector/gpsimd
            # (scalar and sync cannot memset).
            slab_b = src_pool.tile([128, 8192], mybir.dt.float32)
            for r in range(4):
                v = 1.0 if r % 2 == 0 else 0.0
                nc.vector.memset(slab_b[r * 32:(r + 1) * 32, 0:4096], v)
                nc.gpsimd.memset(slab_b[r * 32:(r + 1) * 32, 4096:8192], v)

            # Scatter offsets: [128, 5] int32 flat element indices.
            # Column j: rows 0-63 = ones-half offsets of points
            # 64j..64j+63 (write 0.0), rows 64-127 = z-half offsets of the
            # same points (write 1.0) -- vals_t is just two quadrant-
            # aligned memsets. These queue behind the gpsimd memsets; the
            # scatters need them ~50 us in.
            offs_t = small_pool.tile([128, N_SCATTER_COLS], mybir.dt.int32)
            nc.gpsimd.dma_start(offs_t[:, :], offs[:, :])
            vals_t = small_pool.tile([128, N_SCATTER_COLS], mybir.dt.float32)
            nc.gpsimd.memset(vals_t[0:64, :], 0.0)
            nc.gpsimd.memset(vals_t[64:128, :], 1.0)

            # Fills. sample_fills[s] lists the fills that write slab s.
            #   slabs 0-1: eight 0.5 MB fills from the two constant
            #              quarters (sync: ones quarter -> engines 0-7,
            #              scalar: zeros quarter -> engines 8-15), live at
            #              ~7.4 us
            #   slabs 2-5: 2 MB full-tile stage-A fills (~9 us)
            #   slabs 6-29: twelve 4 MB stage-B fills
            #   slabs 30-31: one single per ring from opposite tile halves
            sample_fills = {s: [] for s in range(BL)}
            Q = HALF // 2
            for s in (0, 1):
                for k in range(2):
                    f = nc.sync.dma_start(
                        out[s:s + 1, k * Q:(k + 1) * Q],
                        slab_a[0:64, 0:2048])
                    sample_fills[s].append(f)
                    f = nc.scalar.dma_start(
                        out[s:s + 1, HALF + k * Q:HALF + (k + 1) * Q],
                        slab_a[64:128, 2048:4096])
                    sample_fills[s].append(f)
            for s in range(2, 6):
                eng = nc.sync if s % 2 == 0 else nc.scalar
                f = eng.dma_start(out[s:s + 1, :], slab_a[:, :])
                sample_fills[s].append(f)
            # Slabs 6-27: 4 MB whole-tile fills -- descriptors are served
            # by the DMA engine owning the source partition group
            # (partition//8), so only fills reading all 128 rows engage
            # all 16 engines on their own (half-row sources measured at
            # exactly half rate when both rings read the same half).
            for i, s in enumerate(range(6, 28, 2)):
                eng = nc.sync if i in (0, 2, 4, 6, 8) else nc.scalar
                f = eng.dma_start(out[s:s + 2, :].flatten(), slab_b[:, :])
                for ss in (s, s + 1):
                    sample_fills[ss].append(f)
            # Slabs 28-31: sixteen 0.5 MB half-half fills sourced from
            # 16-row constant windows of slab_b that EXCLUDE partition
            # groups 0 and 15. Engines 0 and 15 are the chronically slow
            # ones on this chip's even cores (E15/E32/E96/E79 straggle
            # ~30 us in most runs); this trims their byte share ~12% so
            # their uniform-fill work finishes with everyone else.
            # slab_b value map: rows 0-31, 64-95 = 1.0; 32-63, 96-127 = 0.
            ones_w = [(8, 24), (64, 80), (16, 32), (72, 88),
                      (8, 24), (80, 96), (16, 32), (64, 80)]
            zeros_w = [(32, 48), (96, 112), (40, 56), (104, 120),
                       (48, 64), (96, 112), (32, 48), (104, 120)]
            wi = 0
            for s in (28, 29, 30, 31):
                for k in range(2):
                    eng = nc.sync if wi < 6 else nc.scalar
                    o0, o1 = ones_w[wi % 8]
                    f = eng.dma_start(out[s:s + 1, k * Q:(k + 1) * Q],
                                      slab_b[o0:o1, :])
                    sample_fills[s].append(f)
                    z0, z1 = zeros_w[wi % 8]
                    f = eng.dma_start(
                        out[s:s + 1, HALF + k * Q:HALF + (k + 1) * Q],
                        slab_b[z0:z1, :])
                    sample_fills[s].append(f)
                    wi += 1

            # Scatter columns: col j covers points 64j..64j+63, i.e.
            # samples floor(6.4j)..floor(6.4j+6.4).
            def deps(lo, hi):
                seen = []
                for s in range(lo, hi + 1):
                    for f in sample_fills[s]:
                        if f not in seen:
                            seen.append(f)
                return seen
            col_deps = [deps(0, 6), deps(6, 12), deps(12, 19),
                        deps(19, 25), deps(25, 31)]

            # Narrow declared out AP ([1, 1] at offset 0): the real write
            # addresses come from the offset tensor; a full-tensor AP would
            # make Tile serialize every scatter behind every fill (WAW), and
            # the explicit col_deps edges below provide the true ordering.
            out2d = out[0:1, 0:1]
            for j in range(N_SCATTER_COLS):
                sc = nc.gpsimd.indirect_dma_start(
                    out=out2d,
                    out_offset=bass.IndirectOffsetOnAxis(
                        ap=offs_t[:, j:j + 1], axis=0),
                    in_=vals_t[:, j:j + 1],
                    in_offset=None,
                )
                for fl in col_deps[j]:
                    add_dep_helper(sc.ins, fl.ins,
                                   reason="scatter after its sample fills")

    nc.compile()
    return nc


def _compute_indices(coord_v, lows, highs, nmc, L_):
    """Replicates reference.py lines exactly (same jax ops on the default
    device) so the floor/log10 bin boundaries match bit-for-bit."""
    import jax.numpy as jnp

    cv = jnp.asarray(np.asarray(coord_v, dtype=np.float32))
    n = cv.shape[1] // 3
    v10 = cv.at[:, 2::3].set(jnp.log10(cv[:, 2::3]))
    lo = jnp.tile(jnp.asarray(np.asarray(lows, dtype=np.float32)), n)
    hi = jnp.tile(jnp.asarray(np.asarray(highs, dtype=np.float32)), n)
    coord_grid = (v10 - lo) / (hi - lo)
    tr = coord_grid.reshape(-1, 3)
    x_i = jnp.floor(tr[:, 0] * L_).astype(jnp.int32)
    y_i = jnp.floor(tr[:, 1] * L_).astype(jnp.int32)
    m_i = jnp.floor(tr[:, 2] * nmc).astype(jnp.int32)
    return (np.asarray(x_i), np.asarray(y_i), np.asarray(m_i))


def _prepare_in_maps(coord_v, lows, highs, nmc, L):
    nmc = int(nmc)
    L_ = int(L)
    x_i, y_i, m_i = _compute_indices(coord_v, lows, highs, nmc, L_)
    n_batch = coord_v.shape[0]
    n = coord_v.shape[1] // 3
    b_i = np.repeat(np.arange(n_batch, dtype=np.int64), n)

    # Flat element offsets (per core, local slab coordinates).
    flat_ones = ((b_i % BL) * SLAB + m_i.astype(np.int64) * PLANE
                 + y_i.astype(np.int64) * L_ + x_i.astype(np.int64))
    flat_z = flat_ones + HALF

    in_maps = []
    for c in range(NCORES):
        sel = slice(c * PTS, (c + 1) * PTS)
        po = flat_ones[sel]
        pz = flat_z[sel]
        offs_np = np.zeros((128, N_SCATTER_COLS), dtype=np.int32)
        for j in range(N_SCATTER_COLS):
            offs_np[0:64, j] = po[64 * j:64 * (j + 1)]
            offs_np[64:128, j] = pz[64 * j:64 * (j + 1)]
        in_maps.append({"offs": offs_np})
    return in_maps


def _run(in_maps, **kwargs):
    if "nc" not in _CACHE:
        _CACHE["nc"] = _build_nc()
    nc = _CACHE["nc"]
    from concourse.bass_utils import run_bass_kernel_spmd
    return run_bass_kernel_spmd(nc, in_maps, core_ids=list(range(NCORES)),
                                **kwargs)


def kernel(coord_v, lows, highs, nmc, L):
    nmc = int(nmc)
    L_ = int(L)
    assert nmc == NMC and L_ == globals()["L"], (nmc, L_)

    in_maps = _prepare_in_maps(coord_v, lows, highs, nmc, L_)
    res = _run(in_maps)
    parts = [res.results[c]["out"].reshape(BL, 2 * NMC, L_, L_)
             for c in range(NCORES)]
    return np.concatenate(parts, axis=0)


# revision 31
# speedup vs baseline: 1.0928x; 1.0928x over previous
"""Trainium2 Bass kernel for nn_CustomParameterTransform (scatter_memory).

Reference semantics: coord_v [256, 30] holds 10 (x, y, mass) triplets per
sample. Each triplet maps to integer grid indices (x_i, y_i, m_i); a one-hot
volume z [B, 16, 128, 128] is scattered (z[b, m, y, x] = 1) and the output is
concat(1-z, z) over the channel axis -> [256, 32, 128, 128] f32 (512 MB).

Strategy (8 NeuronCores, batch-sharded, no cross-core comm):
  - The output is almost entirely constant (first 16 channels 1.0, last 16
    0.0, except at 640 scatter points per core). Per core: one 64 MB
    write-only region built from SBUF "slab image" tiles whose
    partition-major sweep reproduces whole slabs (alternating 1 MB ones /
    1 MB zeros), so every fill is a contiguous DRAM write and both DMA
    sides stay 2-D (the HWDGE PDMA2D fast path; 3-D/strided APs demote to
    an engine-sequenced slow path measured ~5x slower).
  - Steady-state throughput is capped by the per-core DMA port (~435
    GB/s); a ring dispatches ~4 descriptors/us per outstanding
    instruction, so the ramp is limited by how quickly fill instructions
    become ready. Hence: a [128, 1024] mini tile (one ~0.9 us memset per
    engine) feeds the very first fills, slabs 1-5 are ten 1 MB half-slab
    fills (lots of outstanding instructions early), and slabs 6+ are 4 MB
    fills from a [128, 8192] tile (32 KB rows; rows can't exceed 32 KB
    because a larger slab image needs its value to alternate every <32
    partitions and compute APs must start on 32-partition quadrant
    boundaries).
  - A gpsimd software-DGE fill queue was tried as a third descriptor
    stream and made things worse: engines stall fetching software
    descriptors, throttling the HWDGE rings. Everything stays on the two
    rings.
  - The 640 scatter points are fixed up with indirect (scatter) DMAs whose
    deps are wired to just the fills covering their samples, so all but
    the last column overlap the fill phase.
  - The stock const-AP all-engine barrier in Bass.__init__ is patched out
    (nothing here uses const_aps) and TileContext's epilogue is replaced
    with a light drain, since the event-lowered sem-clear cascade scales
    with instruction count.
  - Indices are computed on the host with the exact same jax ops as the
    reference (bit-identical floor/log10 behavior) and passed per-core as
    a [128, 5] int32 tensor of flat element offsets.
"""

import numpy as np

B = 256
NSRC = 10
NMC = 16
L = 128
NCORES = 8
BL = B // NCORES          # 32 samples per core
PLANE = L * L             # 16384
HALF = NMC * PLANE        # 262144 elements per half-slab
SLAB = 2 * HALF           # 524288 elements per sample
OUT_ELEMS = BL * SLAB     # 16777216 per core (64 MB)

N_SCATTER_COLS = 5        # 640 scatter writes = 128 partitions x 5 columns
PTS = BL * NSRC           # 320 points per core

_CACHE = {}


def _build_nc():
    import concourse.bass as bass
    import concourse.tile as tile
    from concourse import bacc, mybir
    from concourse.tile_rust import add_dep_helper

    import types as _types
    from concourse.vector_clock import ScopedClock

    # The const-AP registration in Bass.__init__ ends with an all-engine
    # barrier (~1.5 us of event-sem chaining at the head of every
    # execution). This kernel never touches const_aps -- memset packs its
    # immediate and the DMAs don't use them -- so elide the barrier for
    # the duration of construction.
    _orig_barrier = bass.Bass.all_engine_barrier
    bass.Bass.all_engine_barrier = lambda self, **kw: None
    try:
        nc = bacc.Bacc("TRN2", target_bir_lowering=False, debug=False,
                       num_devices=NCORES)
    finally:
        bass.Bass.all_engine_barrier = _orig_barrier

    def _light_drain_and_barrier(self, tick_clock, wait_clock):
        """Replaces TileContext._drain_and_barrier for this kernel. The
        stock epilogue is drain + two all-engine EVSEM butterfly barriers
        around the sem clear (~9 us after event lowering). Requirements at
        kernel end are: (1) all DMA completions observed, (2) sems cleared
        for NEFF re-execution, (3) the clear happens after every engine's
        last sem use. (1) is the sync drain's global-clock waits; (3) is a
        counting-sem join (sync arrives only after the drain, so join>=4
        implies all DMA done); (2) is the ranged clear. The second barrier
        is unnecessary: a re-execution cannot start until every engine --
        including the clearing gpsimd -- has ended."""
        nc_ = self.nc
        drain_inst = nc_.sync.drain()
        wait_clock.add_sem_waits(
            drain_inst.ins, ScopedClock({None: tick_clock.global_clock}))
        join = nc_.alloc_semaphore("tail_join")
        for eng in nc_.engines.values():
            if eng is not nc_.gpsimd:
                eng.sem_inc(join, 1)
        n_other = len(nc_.engines) - 1
        nc_.gpsimd.wait_ge(join, n_other)
        popped = nc_._tile_sem_poison_stack.pop()
        assert popped is self._sem_poison
        sems = list(self.sems.allocated().values())
        nc_.clear_and_free_semaphores(sems + [join])

    offs = nc.dram_tensor("offs", [128, N_SCATTER_COLS], mybir.dt.int32,
                          kind="ExternalInput").ap()
    out = nc.dram_tensor("out", [BL, SLAB], mybir.dt.float32,
                         kind="ExternalOutput").ap()

    with tile.TileContext(nc) as tc:
        tc._drain_and_barrier = _types.MethodType(_light_drain_and_barrier, tc)
        with tc.tile_pool(name="src", bufs=1) as src_pool, \
             tc.tile_pool(name="small", bufs=1) as small_pool:
            # Stage A [128, 4096] (1 slab/sweep, 16 KB rows). The first
            # memset on each engine yields one CONSTANT quarter (vector:
            # ones in rows 0-63 cols 0-2048; gpsimd: zeros in rows 64-127
            # cols 2048-4096) at ~7.4 us -- the two rings' first fills
            # read those opposite-partition quarters so both rings start
            # immediately on disjoint engine groups. The full slab image
            # is ready ~9 us.
            slab_a = src_pool.tile([128, 4096], mybir.dt.float32)
            nc.vector.memset(slab_a[0:64, 0:2048], 1.0)
            nc.gpsimd.memset(slab_a[64:128, 2048:4096], 0.0)
            nc.vector.memset(slab_a[64:128, 0:2048], 0.0)
            nc.gpsimd.memset(slab_a[0:64, 2048:4096], 1.0)
            # Stage B [128, 8192] (2 slabs/sweep, 32 KB rows, value
            # alternating every 32 rows), columns split vector/gpsimd
            # (scalar and sync cannot memset).
            slab_b = src_pool.tile([128, 8192], mybir.dt.float32)
            for r in range(4):
                v = 1.0 if r % 2 == 0 else 0.0
                nc.vector.memset(slab_b[r * 32:(r + 1) * 32, 0:4096], v)
                nc.gpsimd.memset(slab_b[r * 32:(r + 1) * 32, 4096:8192], v)

            # Scatter offsets: [128, 5] int32 flat element indices.
            # Column j: rows 0-63 = ones-half offsets of points
            # 64j..64j+63 (write 0.0), rows 64-127 = z-half offsets of the
            # same points (write 1.0) -- vals_t is just two quadrant-
            # aligned memsets. These queue behind the gpsimd memsets; the
            # scatters need them ~50 us in.
            offs_t = small_pool.tile([128, N_SCATTER_COLS], mybir.dt.int32)
            nc.gpsimd.dma_start(offs_t[:, :], offs[:, :])
            vals_t = small_pool.tile([128, N_SCATTER_COLS], mybir.dt.float32)
            nc.gpsimd.memset(vals_t[0:64, :], 0.0)
            nc.gpsimd.memset(vals_t[64:128, :], 1.0)

            # Fills. sample_fills[s] lists the fills that write slab s.
            #   slabs 0-1: eight 0.5 MB fills from the two constant
            #              quarters (sync: ones quarter -> engines 0-7,
            #              scalar: zeros quarter -> engines 8-15), live at
            #              ~7.4 us
            #   slabs 2-5: 2 MB full-tile stage-A fills (~9 us)
            #   slabs 6-29: twelve 4 MB stage-B fills
            #   slabs 30-31: one single per ring from opposite tile halves
            sample_fills = {s: [] for s in range(BL)}
            Q = HALF // 2
            for s in (0, 1):
                for k in range(2):
                    f = nc.sync.dma_start(
                        out[s:s + 1, k * Q:(k + 1) * Q],
                        slab_a[0:64, 0:2048])
                    sample_fills[s].append(f)
                    f = nc.scalar.dma_start(
                        out[s:s + 1, HALF + k * Q:HALF + (k + 1) * Q],
                        slab_a[64:128, 2048:4096])
                    sample_fills[s].append(f)
            for s in range(2, 6):
                eng = nc.sync if s % 2 == 0 else nc.scalar
                f = eng.dma_start(out[s:s + 1, :], slab_a[:, :])
                sample_fills[s].append(f)
            # Slabs 6-29: 4 MB whole-tile fills -- descriptors are served
            # by the DMA engine owning the source partition group
            # (partition//8), so only fills reading all 128 rows engage
            # all 16 engines on their own. Slabs 30-31: one single per
            # ring from opposite tile halves so both rings carry exactly
            # 32 MB with full engine coverage at the tail.
            for i, s in enumerate(range(6, 30, 2)):
                eng = nc.sync if i % 2 == 0 else nc.scalar
                f = eng.dma_start(out[s:s + 2, :].flatten(), slab_b[:, :])
                for ss in (s, s + 1):
                    sample_fills[ss].append(f)
            sample_fills[30].append(
                nc.sync.dma_start(out[30:31, :], slab_b[0:64, :]))
            sample_fills[31].append(
                nc.scalar.dma_start(out[31:32, :], slab_b[64:128, :]))

            # Scatter columns: col j covers points 64j..64j+63, i.e.
            # samples floor(6.4j)..floor(6.4j+6.4).
            def deps(lo, hi):
                seen = []
                for s in range(lo, hi + 1):
                    for f in sample_fills[s]:
                        if f not in seen:
                            seen.append(f)
                return seen
            col_deps = [deps(0, 6), deps(6, 12), deps(12, 19),
                        deps(19, 25), deps(25, 31)]

            # Narrow declared out AP ([1, 1] at offset 0): the real write
            # addresses come from the offset tensor; a full-tensor AP would
            # make Tile serialize every scatter behind every fill (WAW), and
            # the explicit col_deps edges below provide the true ordering.
            out2d = out[0:1, 0:1]
            for j in range(N_SCATTER_COLS):
                sc = nc.gpsimd.indirect_dma_start(
                    out=out2d,
                    out_offset=bass.IndirectOffsetOnAxis(
                        ap=offs_t[:, j:j + 1], axis=0),
                    in_=vals_t[:, j:j + 1],
                    in_offset=None,
                )
                for fl in col_deps[j]:
                    add_dep_helper(sc.ins, fl.ins,
                                   reason="scatter after its sample fills")

    nc.compile()
    return nc


def _compute_indices(coord_v, lows, highs, nmc, L_):
    """Replicates reference.py lines exactly (same jax ops on the default
    device) so the floor/log10 bin boundaries match bit-for-bit."""
    import jax.numpy as jnp

    cv = jnp.asarray(np.asarray(coord_v, dtype=np.float32))
    n = cv.shape[1] // 3
    v10 = cv.at[:, 2::3].set(jnp.log10(cv[:, 2::3]))
    lo = jnp.tile(jnp.asarray(np.asarray(lows, dtype=np.float32)), n)
    hi = jnp.tile(jnp.asarray(np.asarray(highs, dtype=np.float32)), n)
    coord_grid = (v10 - lo) / (hi - lo)
    tr = coord_grid.reshape(-1, 3)
    x_i = jnp.floor(tr[:, 0] * L_).astype(jnp.int32)
    y_i = jnp.floor(tr[:, 1] * L_).astype(jnp.int32)
    m_i = jnp.floor(tr[:, 2] * nmc).astype(jnp.int32)
    return (np.asarray(x_i), np.asarray(y_i), np.asarray(m_i))


def _prepare_in_maps(coord_v, lows, highs, nmc, L):
    nmc = int(nmc)
    L_ = int(L)
    x_i, y_i, m_i = _compute_indices(coord_v, lows, highs, nmc, L_)
    n_batch = coord_v.shape[0]
    n = coord_v.shape[1] // 3
    b_i = np.repeat(np.arange(n_batch, dtype=np.int64), n)

    # Flat element offsets (per core, local slab coordinates).
    flat_ones = ((b_i % BL) * SLAB + m_i.astype(np.int64) * PLANE
                 + y_i.astype(np.int64) * L_ + x_i.astype(np.int64))
    flat_z = flat_ones + HALF

    in_maps = []
    for c in range(NCORES):
        sel = slice(c * PTS, (c + 1) * PTS)
        po = flat_ones[sel]
        pz = flat_z[sel]
        offs_np = np.zeros((128, N_SCATTER_COLS), dtype=np.int32)
        for j in range(N_SCATTER_COLS):
            offs_np[0:64, j] = po[64 * j:64 * (j + 1)]
            offs_np[64:128, j] = pz[64 * j:64 * (j + 1)]
        in_maps.append({"offs": offs_np})
    return in_maps


def _run(in_maps, **kwargs):
    if "nc" not in _CACHE:
        _CACHE["nc"] = _build_nc()
    nc = _CACHE["nc"]
    from concourse.bass_utils import run_bass_kernel_spmd
    return run_bass_kernel_spmd(nc, in_maps, core_ids=list(range(NCORES)),
                                **kwargs)


def kernel(coord_v, lows, highs, nmc, L):
    nmc = int(nmc)
    L_ = int(L)
    assert nmc == NMC and L_ == globals()["L"], (nmc, L_)

    in_maps = _prepare_in_maps(coord_v, lows, highs, nmc, L_)
    res = _run(in_maps)
    parts = [res.results[c]["out"].reshape(BL, 2 * NMC, L_, L_)
             for c in range(NCORES)]
    return np.concatenate(parts, axis=0)


# revision 32
# speedup vs baseline: 1.1038x; 1.0101x over previous
"""Trainium2 Bass kernel for nn_CustomParameterTransform (scatter_memory).

Reference semantics: coord_v [256, 30] holds 10 (x, y, mass) triplets per
sample. Each triplet maps to integer grid indices (x_i, y_i, m_i); a one-hot
volume z [B, 16, 128, 128] is scattered (z[b, m, y, x] = 1) and the output is
concat(1-z, z) over the channel axis -> [256, 32, 128, 128] f32 (512 MB).

Strategy (8 NeuronCores, batch-sharded, no cross-core comm):
  - The output is almost entirely constant (first 16 channels 1.0, last 16
    0.0, except at 640 scatter points per core). Per core: one 64 MB
    write-only region built from SBUF "slab image" tiles whose
    partition-major sweep reproduces whole slabs (alternating 1 MB ones /
    1 MB zeros), so every fill is a contiguous DRAM write and both DMA
    sides stay 2-D (the HWDGE PDMA2D fast path; 3-D/strided APs demote to
    an engine-sequenced slow path measured ~5x slower).
  - Steady-state throughput is capped at ~430 GB/s/core (16 DMA engines
    x ~27 GB/s; descriptors are served by the engine owning the SOURCE
    partition group, partition//8, so full bandwidth requires the active
    fills' source partitions to cover all 128 rows). A ring dispatches
    ~4 descriptors/us per outstanding instruction, so the ramp is limited
    by how quickly fill instructions become ready. Hence: slabs 0-1 are
    eight 0.5 MB fills from the first two memset quarters (opposite
    partition halves on the two rings), slabs 2-5 are full-tile 2 MB
    fills from stage A, and slabs 6+ are 4 MB fills from a [128, 8192]
    stage B (32 KB rows; rows can't exceed 32 KB because a larger slab
    image needs its value to alternate every <32 partitions and
    compute APs must start on 32-partition quadrant boundaries).
  - A gpsimd software-DGE fill queue was tried as a third descriptor
    stream and made things worse: engines stall fetching software
    descriptors, throttling the HWDGE rings. Everything stays on the two
    rings.
  - The 640 scatter points are fixed up with indirect (scatter) DMAs whose
    deps are wired to just the fills covering their samples, so all but
    the last column overlap the fill phase.
  - The stock const-AP all-engine barrier in Bass.__init__ is patched out
    (nothing here uses const_aps) and TileContext's epilogue is replaced
    with a light drain, since the event-lowered sem-clear cascade scales
    with instruction count.
  - Indices are computed on the host with the exact same jax ops as the
    reference (bit-identical floor/log10 behavior) and passed per-core as
    a [128, 5] int32 tensor of flat element offsets.
"""

import numpy as np

B = 256
NSRC = 10
NMC = 16
L = 128
NCORES = 8
BL = B // NCORES          # 32 samples per core
PLANE = L * L             # 16384
HALF = NMC * PLANE        # 262144 elements per half-slab
SLAB = 2 * HALF           # 524288 elements per sample
OUT_ELEMS = BL * SLAB     # 16777216 per core (64 MB)

N_SCATTER_COLS = 5        # 640 scatter writes = 128 partitions x 5 columns
PTS = BL * NSRC           # 320 points per core

_CACHE = {}


def _build_nc():
    import concourse.bass as bass
    import concourse.tile as tile
    from concourse import bacc, mybir
    from concourse.tile_rust import add_dep_helper

    import types as _types
    from concourse.vector_clock import ScopedClock

    # The const-AP registration in Bass.__init__ ends with an all-engine
    # barrier (~1.5 us of event-sem chaining at the head of every
    # execution). This kernel never touches const_aps -- memset packs its
    # immediate and the DMAs don't use them -- so elide the barrier for
    # the duration of construction.
    _orig_barrier = bass.Bass.all_engine_barrier
    bass.Bass.all_engine_barrier = lambda self, **kw: None
    try:
        nc = bacc.Bacc("TRN2", target_bir_lowering=False, debug=False,
                       num_devices=NCORES)
    finally:
        bass.Bass.all_engine_barrier = _orig_barrier

    def _light_drain_and_barrier(self, tick_clock, wait_clock):
        """Replaces TileContext._drain_and_barrier for this kernel. The
        stock epilogue is drain + two all-engine EVSEM butterfly barriers
        around the sem clear (~9 us after event lowering). Requirements at
        kernel end are: (1) all DMA completions observed, (2) sems cleared
        for NEFF re-execution, (3) the clear happens after every engine's
        last sem use. (1) is the sync drain's global-clock waits; (3) is a
        counting-sem join (sync arrives only after the drain, so join>=4
        implies all DMA done); (2) is the ranged clear. The second barrier
        is unnecessary: a re-execution cannot start until every engine --
        including the clearing gpsimd -- has ended."""
        nc_ = self.nc
        drain_inst = nc_.sync.drain()
        wait_clock.add_sem_waits(
            drain_inst.ins, ScopedClock({None: tick_clock.global_clock}))
        join = nc_.alloc_semaphore("tail_join")
        for eng in nc_.engines.values():
            if eng is not nc_.gpsimd:
                eng.sem_inc(join, 1)
        n_other = len(nc_.engines) - 1
        nc_.gpsimd.wait_ge(join, n_other)
        popped = nc_._tile_sem_poison_stack.pop()
        assert popped is self._sem_poison
        sems = list(self.sems.allocated().values())
        nc_.clear_and_free_semaphores(sems + [join])

    offs = nc.dram_tensor("offs", [128, N_SCATTER_COLS], mybir.dt.int32,
                          kind="ExternalInput").ap()
    out = nc.dram_tensor("out", [BL, SLAB], mybir.dt.float32,
                         kind="ExternalOutput").ap()

    with tile.TileContext(nc) as tc:
        tc._drain_and_barrier = _types.MethodType(_light_drain_and_barrier, tc)
        with tc.tile_pool(name="src", bufs=1) as src_pool, \
             tc.tile_pool(name="small", bufs=1) as small_pool:
            # Stage A [128, 4096] (1 slab/sweep, 16 KB rows). The first
            # memset on each engine yields one CONSTANT quarter (vector:
            # ones in rows 0-63 cols 0-2048; gpsimd: zeros in rows 64-127
            # cols 2048-4096) at ~7.4 us -- the two rings' first fills
            # read those opposite-partition quarters so both rings start
            # immediately on disjoint engine groups. The full slab image
            # is ready ~9 us.
            slab_a = src_pool.tile([128, 4096], mybir.dt.float32)
            nc.vector.memset(slab_a[0:64, 0:2048], 1.0)
            nc.gpsimd.memset(slab_a[64:128, 2048:4096], 0.0)
            nc.vector.memset(slab_a[64:128, 0:2048], 0.0)
            nc.gpsimd.memset(slab_a[0:64, 2048:4096], 1.0)
            # Stage B [128, 8192] (2 slabs/sweep, 32 KB rows, value
            # alternating every 32 rows), columns split vector/gpsimd
            # (scalar and sync cannot memset).
            slab_b = src_pool.tile([128, 8192], mybir.dt.float32)
            for r in range(4):
                v = 1.0 if r % 2 == 0 else 0.0
                nc.vector.memset(slab_b[r * 32:(r + 1) * 32, 0:4096], v)
                nc.gpsimd.memset(slab_b[r * 32:(r + 1) * 32, 4096:8192], v)

            # Scatter offsets: [128, 5] int32 flat element indices.
            # Column j: rows 0-63 = ones-half offsets of points
            # 64j..64j+63 (write 0.0), rows 64-127 = z-half offsets of the
            # same points (write 1.0) -- vals_t is just two quadrant-
            # aligned memsets. These queue behind the gpsimd memsets; the
            # scatters need them ~50 us in.
            offs_t = small_pool.tile([128, N_SCATTER_COLS], mybir.dt.int32)
            nc.gpsimd.dma_start(offs_t[:, :], offs[:, :])
            vals_t = small_pool.tile([128, N_SCATTER_COLS], mybir.dt.float32)
            nc.gpsimd.memset(vals_t[0:64, :], 0.0)
            nc.gpsimd.memset(vals_t[64:128, :], 1.0)

            # Fills. sample_fills[s] lists the fills that write slab s.
            #   slabs 0-1: eight 0.5 MB fills from the two constant
            #              quarters (sync: ones quarter -> engines 0-7,
            #              scalar: zeros quarter -> engines 8-15), live at
            #              ~7.4 us
            #   slabs 2-5: 2 MB full-tile stage-A fills (~9 us)
            #   slabs 6-29: twelve 4 MB stage-B fills
            #   slabs 30-31: one single per ring from opposite tile halves
            sample_fills = {s: [] for s in range(BL)}
            Q = HALF // 2
            for s in (0, 1):
                for k in range(2):
                    f = nc.sync.dma_start(
                        out[s:s + 1, k * Q:(k + 1) * Q],
                        slab_a[0:64, 0:2048])
                    sample_fills[s].append(f)
                    f = nc.scalar.dma_start(
                        out[s:s + 1, HALF + k * Q:HALF + (k + 1) * Q],
                        slab_a[64:128, 2048:4096])
                    sample_fills[s].append(f)
            for s in range(2, 6):
                eng = nc.sync if s % 2 == 0 else nc.scalar
                f = eng.dma_start(out[s:s + 1, :], slab_a[:, :])
                sample_fills[s].append(f)
            # Slabs 6-29: 4 MB whole-tile fills -- descriptors are served
            # by the DMA engine owning the source partition group
            # (partition//8), so only fills reading all 128 rows engage
            # all 16 engines on their own. Slabs 30-31: one single per
            # ring from opposite tile halves so both rings carry exactly
            # 32 MB with full engine coverage at the tail.
            for i, s in enumerate(range(6, 30, 2)):
                eng = nc.sync if i % 2 == 0 else nc.scalar
                f = eng.dma_start(out[s:s + 2, :].flatten(), slab_b[:, :])
                for ss in (s, s + 1):
                    sample_fills[ss].append(f)
            sample_fills[30].append(
                nc.sync.dma_start(out[30:31, :], slab_b[0:64, :]))
            sample_fills[31].append(
                nc.scalar.dma_start(out[31:32, :], slab_b[64:128, :]))

            # Scatter columns: col j covers points 64j..64j+63, i.e.
            # samples floor(6.4j)..floor(6.4j+6.4).
            def deps(lo, hi):
                seen = []
                for s in range(lo, hi + 1):
                    for f in sample_fills[s]:
                        if f not in seen:
                            seen.append(f)
                return seen
            col_deps = [deps(0, 6), deps(6, 12), deps(12, 19),
                        deps(19, 25), deps(25, 31)]

            # Narrow declared out AP ([1, 1] at offset 0): the real write
            # addresses come from the offset tensor; a full-tensor AP would
            # make Tile serialize every scatter behind every fill (WAW), and
            # the explicit col_deps edges below provide the true ordering.
            out2d = out[0:1, 0:1]
            for j in range(N_SCATTER_COLS):
                sc = nc.gpsimd.indirect_dma_start(
                    out=out2d,
                    out_offset=bass.IndirectOffsetOnAxis(
                        ap=offs_t[:, j:j + 1], axis=0),
                    in_=vals_t[:, j:j + 1],
                    in_offset=None,
                )
                for fl in col_deps[j]:
                    add_dep_helper(sc.ins, fl.ins,
                                   reason="scatter after its sample fills")

    nc.compile()
    return nc


def _compute_indices(coord_v, lows, highs, nmc, L_):
    """Replicates reference.py lines exactly (same jax ops on the default
    device) so the floor/log10 bin boundaries match bit-for-bit."""
    import jax.numpy as jnp

    cv = jnp.asarray(np.asarray(coord_v, dtype=np.float32))
    n = cv.shape[1] // 3
    v10 = cv.at[:, 2::3].set(jnp.log10(cv[:, 2::3]))
    lo = jnp.tile(jnp.asarray(np.asarray(lows, dtype=np.float32)), n)
    hi = jnp.tile(jnp.asarray(np.asarray(highs, dtype=np.float32)), n)
    coord_grid = (v10 - lo) / (hi - lo)
    tr = coord_grid.reshape(-1, 3)
    x_i = jnp.floor(tr[:, 0] * L_).astype(jnp.int32)
    y_i = jnp.floor(tr[:, 1] * L_).astype(jnp.int32)
    m_i = jnp.floor(tr[:, 2] * nmc).astype(jnp.int32)
    return (np.asarray(x_i), np.asarray(y_i), np.asarray(m_i))


def _prepare_in_maps(coord_v, lows, highs, nmc, L):
    nmc = int(nmc)
    L_ = int(L)
    x_i, y_i, m_i = _compute_indices(coord_v, lows, highs, nmc, L_)
    n_batch = coord_v.shape[0]
    n = coord_v.shape[1] // 3
    b_i = np.repeat(np.arange(n_batch, dtype=np.int64), n)

    # Flat element offsets (per core, local slab coordinates).
    flat_ones = ((b_i % BL) * SLAB + m_i.astype(np.int64) * PLANE
                 + y_i.astype(np.int64) * L_ + x_i.astype(np.int64))
    flat_z = flat_ones + HALF

    in_maps = []
    for c in range(NCORES):
        sel = slice(c * PTS, (c + 1) * PTS)
        po = flat_ones[sel]
        pz = flat_z[sel]
        offs_np = np.zeros((128, N_SCATTER_COLS), dtype=np.int32)
        for j in range(N_SCATTER_COLS):
            offs_np[0:64, j] = po[64 * j:64 * (j + 1)]
            offs_np[64:128, j] = pz[64 * j:64 * (j + 1)]
        in_maps.append({"offs": offs_np})
    return in_maps


def _run(in_maps, **kwargs):
    if "nc" not in _CACHE:
        _CACHE["nc"] = _build_nc()
    nc = _CACHE["nc"]
    from concourse.bass_utils import run_bass_kernel_spmd
    return run_bass_kernel_spmd(nc, in_maps, core_ids=list(range(NCORES)),
                                **kwargs)


def kernel(coord_v, lows, highs, nmc, L):
    nmc = int(nmc)
    L_ = int(L)
    assert nmc == NMC and L_ == globals()["L"], (nmc, L_)

    in_maps = _prepare_in_maps(coord_v, lows, highs, nmc, L_)
    res = _run(in_maps)
    parts = [res.results[c]["out"].reshape(BL, 2 * NMC, L_, L_)
             for c in range(NCORES)]
    return np.concatenate(parts, axis=0)


# revision 34
# speedup vs baseline: 1.1047x; 1.0008x over previous
"""Trainium2 Bass kernel for nn_CustomParameterTransform (scatter_memory).

Reference semantics: coord_v [256, 30] holds 10 (x, y, mass) triplets per
sample. Each triplet maps to integer grid indices (x_i, y_i, m_i); a one-hot
volume z [B, 16, 128, 128] is scattered (z[b, m, y, x] = 1) and the output is
concat(1-z, z) over the channel axis -> [256, 32, 128, 128] f32 (512 MB).

Strategy (8 NeuronCores, batch-sharded, no cross-core comm):
  - The output is almost entirely constant (first 16 channels 1.0, last 16
    0.0, except at 640 scatter points per core). Per core: one 64 MB
    write-only region built from SBUF "slab image" tiles whose
    partition-major sweep reproduces whole slabs (alternating 1 MB ones /
    1 MB zeros), so every fill is a contiguous DRAM write and both DMA
    sides stay 2-D (the HWDGE PDMA2D fast path; 3-D/strided APs demote to
    an engine-sequenced slow path measured ~5x slower).
  - Steady-state throughput is capped at ~430 GB/s/core (16 DMA engines
    x ~27 GB/s; descriptors are served by the engine owning the SOURCE
    partition group, partition//8, so full bandwidth requires the active
    fills' source partitions to cover all 128 rows). A ring dispatches
    ~4 descriptors/us per outstanding instruction, so the ramp is limited
    by how quickly fill instructions become ready. Hence: slabs 0-1 are
    eight 0.5 MB fills from the first two memset quarters (opposite
    partition halves on the two rings), slabs 2-5 are full-tile 2 MB
    fills from stage A, and slabs 6+ are 4 MB fills from a [128, 8192]
    stage B (32 KB rows; rows can't exceed 32 KB because a larger slab
    image needs its value to alternate every <32 partitions and
    compute APs must start on 32-partition quadrant boundaries).
  - A gpsimd software-DGE fill queue was tried as a third descriptor
    stream and made things worse: engines stall fetching software
    descriptors, throttling the HWDGE rings. Everything stays on the two
    rings.
  - The 640 scatter points are fixed up with indirect (scatter) DMAs whose
    deps are wired to just the fills covering their samples, so all but
    the last column overlap the fill phase.
  - The stock const-AP all-engine barrier in Bass.__init__ is patched out
    (nothing here uses const_aps) and TileContext's epilogue is replaced
    with a light drain, since the event-lowered sem-clear cascade scales
    with instruction count.
  - Indices are computed on the host with the exact same jax ops as the
    reference (bit-identical floor/log10 behavior) and passed per-core as
    a [128, 5] int32 tensor of flat element offsets.
"""

import numpy as np

B = 256
NSRC = 10
NMC = 16
L = 128
NCORES = 8
BL = B // NCORES          # 32 samples per core
PLANE = L * L             # 16384
HALF = NMC * PLANE        # 262144 elements per half-slab
SLAB = 2 * HALF           # 524288 elements per sample
OUT_ELEMS = BL * SLAB     # 16777216 per core (64 MB)

N_SCATTER_COLS = 5        # 640 scatter writes = 128 partitions x 5 columns
PTS = BL * NSRC           # 320 points per core

_CACHE = {}


def _build_nc():
    import concourse.bass as bass
    import concourse.tile as tile
    from concourse import bacc, mybir
    from concourse.tile_rust import add_dep_helper

    import types as _types
    from concourse.vector_clock import ScopedClock

    # The const-AP registration in Bass.__init__ ends with an all-engine
    # barrier (~1.5 us of event-sem chaining at the head of every
    # execution). This kernel never touches const_aps -- memset packs its
    # immediate and the DMAs don't use them -- so elide the barrier for
    # the duration of construction.
    _orig_barrier = bass.Bass.all_engine_barrier
    bass.Bass.all_engine_barrier = lambda self, **kw: None
    try:
        nc = bacc.Bacc("TRN2", target_bir_lowering=False, debug=False,
                       num_devices=NCORES)
    finally:
        bass.Bass.all_engine_barrier = _orig_barrier

    def _light_drain_and_barrier(self, tick_clock, wait_clock):
        """Replaces TileContext._drain_and_barrier for this kernel. The
        stock epilogue is drain + two all-engine EVSEM butterfly barriers
        around the sem clear (~9 us after event lowering). Requirements at
        kernel end are: (1) all DMA completions observed, (2) sems cleared
        for NEFF re-execution, (3) the clear happens after every engine's
        last sem use. (1) is the sync drain's global-clock waits; (3) is a
        counting-sem join (sync arrives only after the drain, so join>=4
        implies all DMA done); (2) is the ranged clear. The second barrier
        is unnecessary: a re-execution cannot start until every engine --
        including the clearing gpsimd -- has ended."""
        nc_ = self.nc
        drain_inst = nc_.sync.drain()
        wait_clock.add_sem_waits(
            drain_inst.ins, ScopedClock({None: tick_clock.global_clock}))
        join = nc_.alloc_semaphore("tail_join")
        for eng in nc_.engines.values():
            if eng is not nc_.gpsimd:
                eng.sem_inc(join, 1)
        n_other = len(nc_.engines) - 1
        nc_.gpsimd.wait_ge(join, n_other)
        popped = nc_._tile_sem_poison_stack.pop()
        assert popped is self._sem_poison
        sems = list(self.sems.allocated().values())
        nc_.clear_and_free_semaphores(sems + [join])

    offs = nc.dram_tensor("offs", [128, N_SCATTER_COLS], mybir.dt.int32,
                          kind="ExternalInput").ap()
    out = nc.dram_tensor("out", [BL, SLAB], mybir.dt.float32,
                         kind="ExternalOutput").ap()

    with tile.TileContext(nc) as tc:
        tc._drain_and_barrier = _types.MethodType(_light_drain_and_barrier, tc)
        with tc.tile_pool(name="src", bufs=1) as src_pool, \
             tc.tile_pool(name="small", bufs=1) as small_pool:
            # Mini tiles [128, 1024]: the first memset on each engine
            # (~0.9 us), full 128-partition sources so even the first
            # fills engage all 16 DMA engines. They feed eight 0.5 MB
            # fills covering slabs 0-1 from ~7.3 us.
            ones_mini = src_pool.tile([128, 1024], mybir.dt.float32)
            zeros_mini = src_pool.tile([128, 1024], mybir.dt.float32)
            nc.vector.memset(ones_mini[:, :], 1.0)
            nc.gpsimd.memset(zeros_mini[:, :], 0.0)
            # Stage A [128, 4096] (1 slab/sweep, 16 KB rows), ready ~11 us.
            slab_a = src_pool.tile([128, 4096], mybir.dt.float32)
            nc.vector.memset(slab_a[0:64, 0:2048], 1.0)
            nc.gpsimd.memset(slab_a[64:128, 2048:4096], 0.0)
            nc.vector.memset(slab_a[64:128, 0:2048], 0.0)
            nc.gpsimd.memset(slab_a[0:64, 2048:4096], 1.0)
            # Stage B [128, 8192] (2 slabs/sweep, 32 KB rows, value
            # alternating every 32 rows), columns split vector/gpsimd
            # (scalar and sync cannot memset).
            slab_b = src_pool.tile([128, 8192], mybir.dt.float32)
            for r in range(4):
                v = 1.0 if r % 2 == 0 else 0.0
                nc.vector.memset(slab_b[r * 32:(r + 1) * 32, 0:4096], v)
                nc.gpsimd.memset(slab_b[r * 32:(r + 1) * 32, 4096:8192], v)

            # Scatter offsets: [128, 5] int32 flat element indices.
            # Column j: rows 0-63 = ones-half offsets of points
            # 64j..64j+63 (write 0.0), rows 64-127 = z-half offsets of the
            # same points (write 1.0) -- vals_t is just two quadrant-
            # aligned memsets. These queue behind the gpsimd memsets; the
            # scatters need them ~50 us in.
            offs_t = small_pool.tile([128, N_SCATTER_COLS], mybir.dt.int32)
            nc.gpsimd.dma_start(offs_t[:, :], offs[:, :])
            vals_t = small_pool.tile([128, N_SCATTER_COLS], mybir.dt.float32)
            nc.gpsimd.memset(vals_t[0:64, :], 0.0)
            nc.gpsimd.memset(vals_t[64:128, :], 1.0)

            # Fills. sample_fills[s] lists the fills that write slab s.
            #   slabs 0-1: eight 0.5 MB mini fills (sync: ones halves,
            #              scalar: zeros halves; each ring's sem wait is
            #              just its own mini memset), live ~7.3 us
            #   slabs 2-5: 2 MB full-tile stage-A fills (~11 us)
            #   slabs 6-29: twelve 4 MB stage-B fills
            #   slabs 30-31: one single per ring from opposite tile halves
            sample_fills = {s: [] for s in range(BL)}
            Q = HALF // 2
            for s in (0, 1):
                for k in range(2):
                    f = nc.sync.dma_start(
                        out[s:s + 1, k * Q:(k + 1) * Q], ones_mini[:, :])
                    sample_fills[s].append(f)
                    f = nc.scalar.dma_start(
                        out[s:s + 1, HALF + k * Q:HALF + (k + 1) * Q],
                        zeros_mini[:, :])
                    sample_fills[s].append(f)
            for s in range(2, 6):
                eng = nc.sync if s % 2 == 0 else nc.scalar
                f = eng.dma_start(out[s:s + 1, :], slab_a[:, :])
                sample_fills[s].append(f)
            # Slabs 6-29: 4 MB whole-tile fills -- descriptors are served
            # by the DMA engine owning the source partition group
            # (partition//8), so only fills reading all 128 rows engage
            # all 16 engines on their own. Slabs 30-31: one single per
            # ring from opposite tile halves so both rings carry exactly
            # 32 MB with full engine coverage at the tail.
            for i, s in enumerate(range(6, 30, 2)):
                eng = nc.sync if i % 2 == 0 else nc.scalar
                f = eng.dma_start(out[s:s + 2, :].flatten(), slab_b[:, :])
                for ss in (s, s + 1):
                    sample_fills[ss].append(f)
            sample_fills[30].append(
                nc.sync.dma_start(out[30:31, :], slab_b[0:64, :]))
            sample_fills[31].append(
                nc.scalar.dma_start(out[31:32, :], slab_b[64:128, :]))

            # Scatter columns: col j covers points 64j..64j+63, i.e.
            # samples floor(6.4j)..floor(6.4j+6.4).
            def deps(lo, hi):
                seen = []
                for s in range(lo, hi + 1):
                    for f in sample_fills[s]:
                        if f not in seen:
                            seen.append(f)
                return seen
            col_deps = [deps(0, 6), deps(6, 12), deps(12, 19),
                        deps(19, 25), deps(25, 31)]

            # Narrow declared out AP ([1, 1] at offset 0): the real write
            # addresses come from the offset tensor; a full-tensor AP would
            # make Tile serialize every scatter behind every fill (WAW), and
            # the explicit col_deps edges below provide the true ordering.
            out2d = out[0:1, 0:1]
            for j in range(N_SCATTER_COLS):
                sc = nc.gpsimd.indirect_dma_start(
                    out=out2d,
                    out_offset=bass.IndirectOffsetOnAxis(
                        ap=offs_t[:, j:j + 1], axis=0),
                    in_=vals_t[:, j:j + 1],
                    in_offset=None,
                )
                for fl in col_deps[j]:
                    add_dep_helper(sc.ins, fl.ins,
                                   reason="scatter after its sample fills")

    nc.compile()
    return nc


def _compute_indices(coord_v, lows, highs, nmc, L_):
    """Replicates reference.py lines exactly (same jax ops on the default
    device) so the floor/log10 bin boundaries match bit-for-bit."""
    import jax.numpy as jnp

    cv = jnp.asarray(np.asarray(coord_v, dtype=np.float32))
    n = cv.shape[1] // 3
    v10 = cv.at[:, 2::3].set(jnp.log10(cv[:, 2::3]))
    lo = jnp.tile(jnp.asarray(np.asarray(lows, dtype=np.float32)), n)
    hi = jnp.tile(jnp.asarray(np.asarray(highs, dtype=np.float32)), n)
    coord_grid = (v10 - lo) / (hi - lo)
    tr = coord_grid.reshape(-1, 3)
    x_i = jnp.floor(tr[:, 0] * L_).astype(jnp.int32)
    y_i = jnp.floor(tr[:, 1] * L_).astype(jnp.int32)
    m_i = jnp.floor(tr[:, 2] * nmc).astype(jnp.int32)
    return (np.asarray(x_i), np.asarray(y_i), np.asarray(m_i))


def _prepare_in_maps(coord_v, lows, highs, nmc, L):
    nmc = int(nmc)
    L_ = int(L)
    x_i, y_i, m_i = _compute_indices(coord_v, lows, highs, nmc, L_)
    n_batch = coord_v.shape[0]
    n = coord_v.shape[1] // 3
    b_i = np.repeat(np.arange(n_batch, dtype=np.int64), n)

    # Flat element offsets (per core, local slab coordinates).
    flat_ones = ((b_i % BL) * SLAB + m_i.astype(np.int64) * PLANE
                 + y_i.astype(np.int64) * L_ + x_i.astype(np.int64))
    flat_z = flat_ones + HALF

    in_maps = []
    for c in range(NCORES):
        sel = slice(c * PTS, (c + 1) * PTS)
        po = flat_ones[sel]
        pz = flat_z[sel]
        offs_np = np.zeros((128, N_SCATTER_COLS), dtype=np.int32)
        for j in range(N_SCATTER_COLS):
            offs_np[0:64, j] = po[64 * j:64 * (j + 1)]
            offs_np[64:128, j] = pz[64 * j:64 * (j + 1)]
        in_maps.append({"offs": offs_np})
    return in_maps


def _run(in_maps, **kwargs):
    if "nc" not in _CACHE:
        _CACHE["nc"] = _build_nc()
    nc = _CACHE["nc"]
    from concourse.bass_utils import run_bass_kernel_spmd
    return run_bass_kernel_spmd(nc, in_maps, core_ids=list(range(NCORES)),
                                **kwargs)


def kernel(coord_v, lows, highs, nmc, L):
    nmc = int(nmc)
    L_ = int(L)
    assert nmc == NMC and L_ == globals()["L"], (nmc, L_)

    in_maps = _prepare_in_maps(coord_v, lows, highs, nmc, L_)
    res = _run(in_maps)
    parts = [res.results[c]["out"].reshape(BL, 2 * NMC, L_, L_)
             for c in range(NCORES)]
    return np.concatenate(parts, axis=0)
